# revision 49
# baseline (speedup 1.0000x reference)
"""Trainium2 Bass kernel for nn_Attention_55087250538754.

Pre-LN single-head attention block: LayerNorm -> qkv proj -> RoPE(q,k) ->
MultiheadAttention in_proj -> softmax attention -> out_proj.

Sharding: 8 cores = (batch, seq-half). Core c = 2*b + h computes queries,
keys and values for its own half [h*2048, (h+1)*2048) of batch b, then the
two cores of each batch exchange K/V halves with per-block pair-wise
AllGather collectives (sequence-parallel attention; the gathers pipeline
under the projection compute).

Device layout is transposed [feature, row] throughout so every matmul's
contraction dim sits on SBUF partitions. Host-side (input-independent
or O(d^2)/O(S*d) weight-fold) precomputation:
  - x transposed (bf16) and rolled so each core's query rows are local rows
    0..2047
  - ln_g folded into qkv_w; ln_b/qkv_b folded into a per-output-channel
    constant cb added during the qkv PSUM eviction
  - rope: rope(q) = q*cos + (q*sin) @ R.T with R the pair-rotation matrix;
    R is folded into the in_proj weights (wq@R, wk@R), so rope costs two
    elementwise table multiplies fused into the qkv PSUM evictions
  - rope of the pre-rope bias vector (position-dependent) lands in additive
    tables TQ/TK applied during the in_proj eviction
Softmax: scores are tiny (|s| < 1) so exp needs no max subtraction; the
normalization divides the PV output via a row-sum computed with a
ones-vector matmul.

Schedule: the LN stats for block i are computed one iteration ahead of the
block's matmuls; the per-row mean/rsig rows are broadcast across partitions
with K=1 ones-matmuls straight into PSUM, so no block waits on a serial
stats chain or a DRAM roundtrip. Phase D
interleaves each q-tile's softmax/out_proj tail with the next q-tile's
score matmuls to keep TensorE dense.
"""

import math

import numpy as np
import ml_dtypes

import concourse.bass as bass
import concourse.mybir as mybir
import concourse.tile as tile
from concourse import bacc
from concourse.bass_utils import run_bass_kernel_spmd

BF16 = ml_dtypes.bfloat16

D = 512
B = 4
S = 4096
SQ = S // 2          # query rows per core
N_CORES = 8
RB = 512             # r-block (column) size for phases A-C
NB = S // RB
NBQ = SQ // RB       # r-blocks that carry a query half
NKC = S // 128       # 32 key chunks
NBL = SQ // RB       # 4 local r-blocks (own half only; K/V halves exchanged)
RG = [[0, 1], [2, 3], [4, 5], [6, 7]]  # seq-half pairs per batch
NQT = SQ // 512      # 4 query tiles in phase D
DT = mybir.dt
ADD = mybir.AluOpType.add
MULT = mybir.AluOpType.mult
SUB = mybir.AluOpType.subtract


def _bcast_ap(src_ap, n=128):
    """AP re-reading a row n times via a step-0 dim (DMA broadcast source)."""
    return bass.AP(tensor=src_ap.tensor, offset=src_ap.offset,
                   ap=[list(src_ap.ap[0]), [0, n]] + [list(a) for a in src_ap.ap[1:]])


def _mm_acc(nc, ps, lhsT_tiles, rhs_tiles):
    n = len(lhsT_tiles)
    for i, (lh, rh) in enumerate(zip(lhsT_tiles, rhs_tiles)):
        nc.tensor.matmul(ps, lh, rh, start=(i == 0), stop=(i == n - 1))


def build_nc():
    nc = bacc.Bacc()

    # inputs are packed partition-major on the host (see _pack/_packw) so
    # every DMA moves multi-KB contiguous runs per partition
    xT = nc.declare_dram_parameter("xT", [128, NBL * 4 * RB], DT.bfloat16,
                                   isOutput=False)
    cosT = nc.declare_dram_parameter("cosT", [128, NBL * 4 * RB], DT.bfloat16,
                                     isOutput=False)
    sinT = nc.declare_dram_parameter("sinT", [128, NBL * 4 * RB], DT.bfloat16,
                                     isOutput=False)
    wgT = nc.declare_dram_parameter("wgT", [128, 4 * 3 * D], DT.bfloat16,
                                    isOutput=False)
    wqT = nc.declare_dram_parameter("wqT", [128, 8 * D], DT.bfloat16,
                                    isOutput=False)
    wkT = nc.declare_dram_parameter("wkT", [128, 8 * D], DT.bfloat16,
                                    isOutput=False)
    wvT = nc.declare_dram_parameter("wvT", [128, 4 * D], DT.bfloat16,
                                    isOutput=False)
    woT = nc.declare_dram_parameter("woT", [128, 4 * D], DT.bfloat16,
                                    isOutput=False)
    tq = nc.declare_dram_parameter("tq", [128, NBL * 4 * RB], DT.bfloat16,
                                   isOutput=False)
    tk = nc.declare_dram_parameter("tk", [128, NBL * 4 * RB], DT.bfloat16,
                                   isOutput=False)
    cb = nc.declare_dram_parameter("cb", [128, 12], DT.float32, isOutput=False)
    outb = nc.declare_dram_parameter("outb", [128, 4], DT.float32, isOutput=False)
    cv = nc.declare_dram_parameter("cv", [128, D], DT.float32, isOutput=False)
    out = nc.declare_dram_parameter("out", [D, SQ], DT.float32, isOutput=True)



    with tile.TileContext(nc) as tc:
        with tc.tile_pool(name="weights", bufs=1) as wp, \
             tc.tile_pool(name="persist", bufs=1) as pp:
            # --- weights, loaded once ---
            wg_t = wp.tile([128, 4, 3 * D], DT.bfloat16)
            wq_t = wp.tile([128, 8, D], DT.bfloat16)
            wk_t = wp.tile([128, 8, D], DT.bfloat16)
            wv_t = wp.tile([128, 4, D], DT.bfloat16)
            wo_t = wp.tile([128, 4, D], DT.bfloat16)
            cb_t = wp.tile([128, 12], DT.float32)
            outb_t = wp.tile([128, 4], DT.float32)
            cv_t = wp.tile([128, D], DT.float32)
            ones_bf = wp.tile([128, 1], DT.bfloat16)
            ones_k1 = wp.tile([1, 128], DT.bfloat16)
            eps_t = wp.tile([128, 1], DT.float32)
            nc.vector.memset(eps_t[:], 1e-5)
            nc.vector.memset(ones_bf[:], 1.0)
            nc.vector.memset(ones_k1[:], 1.0)

            def emit_weight_loads():
                nc.sync.dma_start(out=wg_t[:], in_=wgT[:])
                nc.sync.dma_start(out=wv_t[:], in_=wvT[:])
                nc.sync.dma_start(out=wo_t[:], in_=woT[:])
                nc.sync.dma_start(out=wq_t[:], in_=wqT[:])
                nc.sync.dma_start(out=wk_t[:], in_=wkT[:])
                nc.sync.dma_start(out=cb_t[:], in_=cb[:])
                nc.sync.dma_start(out=outb_t[:], in_=outb[:])
                nc.sync.dma_start(out=cv_t[:], in_=cv[:])

            # --- persistent activations ---
            q2_t = pp.tile([128, 4, SQ], DT.bfloat16)
            k2_t = pp.tile([128, 4, S], DT.bfloat16)
            v2_t = pp.tile([128, NKC, D], DT.bfloat16)

            # -------- phases A-C: LN stats / qkv+rope / in_proj -----------
            # One loop, staggered: iteration `it` emits the LN-stats part for
            # block `it` and the main part (center, qkv, rope, in_proj) for
            # block `it-1`, whose stats round-tripped through DRAM and come
            # back via step-0 broadcast DMAs. This keeps no serial stats
            # chain in front of any block's matmuls.
            with tc.tile_pool(name="blk", bufs=3) as bp, \
                 tc.tile_pool(name="blk2", bufs=2) as bp2, \
                 tc.tile_pool(name="blk1", bufs=1) as bp1, \
                 tc.tile_pool(name="rope", bufs=2) as rp, \
                 tc.tile_pool(name="rope1", bufs=1) as rp1, \
                 tc.tile_pool(name="stg", bufs=1) as stg, \
                 tc.tile_pool(name="ps_mm", bufs=4, space="PSUM") as mmp, \
                 tc.tile_pool(name="ps_stat", bufs=1, space="PSUM") as stp, \
                 tc.tile_pool(name="ps_bc", bufs=1, space="PSUM") as bcp:
                kv_in = nc.dram_tensor("kv_in", [NBL, 2, D * RB], DT.bfloat16)
                kv_out = nc.dram_tensor("kv_out", [NBL, 4, D * RB], DT.bfloat16)
                xs = {}
                rows = {}

                def emit_stats(rb):
                    r0 = rb * RB
                    x_blk = bp.tile([128, 4, RB], DT.bfloat16, tag="x", name="x_blk")
                    xs[rb] = x_blk
                    nc.scalar.dma_start(out=x_blk[:], in_=xT[:, rb * 4 * RB:(rb + 1) * 4 * RB])
                    xsq_blk = bp1.tile([128, 4, RB], DT.bfloat16, tag="xsq",
                                       name="xsq_blk")
                    for c in range(4):
                        nc.vector.tensor_mul(
                            xsq_blk[:, c, :], x_blk[:, c, :], x_blk[:, c, :])
                    mu_ps = stp.tile([1, RB], DT.float32, tag="mu", name="mu_ps")
                    sq_ps = stp.tile([1, RB], DT.float32, tag="sq", name="sq_ps")
                    _mm_acc(nc, mu_ps[:], [ones_bf[:]] * 4,
                            [x_blk[:, c, :] for c in range(4)])
                    _mm_acc(nc, sq_ps[:], [ones_bf[:]] * 4,
                            [xsq_blk[:, c, :] for c in range(4)])
                    mu_row = bp1.tile([1, RB], DT.float32, tag="mu_row",
                                      name="mu_row")
                    var_row = bp1.tile([1, RB], DT.float32, tag="var_row",
                                       name="var_row")
                    rsig_row = bp1.tile([1, RB], DT.float32, tag="rsig_row",
                                        name="rsig_row")
                    rows_bf = bp2.tile([1, 2, RB], DT.bfloat16, tag="rows_bf",
                                       name="rows_bf")
                    rows[rb] = rows_bf
                    nc.vector.tensor_scalar(mu_row[:], mu_ps[:], 1.0 / D, None, MULT)
                    nc.vector.tensor_scalar(var_row[:], sq_ps[:], 1.0 / D, None, MULT)
                    nc.vector.tensor_mul(rsig_row[:], mu_row[:], mu_row[:])
                    nc.vector.tensor_sub(var_row[:], var_row[:], rsig_row[:])
                    nc.scalar.activation(var_row[:], var_row[:],
                                         mybir.ActivationFunctionType.Sqrt,
                                         bias=eps_t[0:1, :], scale=1.0)
                    nc.vector.reciprocal(rsig_row[:], var_row[:])
                    nc.vector.tensor_copy(rows_bf[:, 0, :], mu_row[:])
                    nc.vector.tensor_copy(rows_bf[:, 1, :], rsig_row[:])


                def emit_main(rb):
                    r0 = rb * RB
                    x_blk = xs.pop(rb)
                    rows_bf = rows.pop(rb)
                    mu_bc = bcp.tile([128, RB], DT.float32, tag="mu_bc",
                                     name="mu_bc")
                    rsig_bc = bcp.tile([128, RB], DT.float32, tag="rsig_bc",
                                       name="rsig_bc")
                    nc.tensor.matmul(mu_bc[:], ones_k1[:], rows_bf[:, 0, :],
                                     start=True, stop=True)
                    nc.tensor.matmul(rsig_bc[:], ones_k1[:], rows_bf[:, 1, :],
                                     start=True, stop=True)
                    xn_blk = rp.tile([128, 4, RB], DT.bfloat16, tag="xn",
                                     name="xn_blk")
                    for c in range(4):
                        nc.vector.tensor_sub(xn_blk[:, c, :], x_blk[:, c, :], mu_bc[:])
                        nc.vector.tensor_mul(xn_blk[:, c, :], xn_blk[:, c, :],
                                             rsig_bc[:])

                    cos_blk = bp1.tile([128, 4, RB], DT.bfloat16, tag="cos",
                                       name="cos_blk")
                    sin_blk = bp1.tile([128, 4, RB], DT.bfloat16, tag="sin",
                                       name="sin_blk")
                    tk_blk = bp1.tile([128, 4, RB], DT.bfloat16, tag="tk",
                                      name="tk_blk")
                    nc.gpsimd.dma_start(out=cos_blk[:], in_=cosT[:, rb * 4 * RB:(rb + 1) * 4 * RB])
                    nc.gpsimd.dma_start(out=sin_blk[:], in_=sinT[:, rb * 4 * RB:(rb + 1) * 4 * RB])
                    nc.gpsimd.dma_start(out=tk_blk[:], in_=tk[:, rb * 4 * RB:(rb + 1) * 4 * RB])
                    tq_blk = bp1.tile([128, 4, RB], DT.bfloat16, tag="tq",
                                      name="tq_blk")
                    nc.gpsimd.dma_start(out=tq_blk[:], in_=tq[:, rb * 4 * RB:(rb + 1) * 4 * RB])

                    # qkv matmuls + fused rope/bias evictions
                    qrope = rp.tile([128, 8, RB], DT.bfloat16, tag="qrope",
                                    name="qrope")
                    krope = rp1.tile([128, 8, RB], DT.bfloat16, tag="krope",
                                    name="krope")
                    vn_blk = rp1.tile([128, 4, RB], DT.bfloat16, tag="vn",
                                      name="vn_blk")
                    for ot in range(12):
                        is_q = ot < 4
                        ps = mmp.tile([128, RB], DT.float32, tag="mm")
                        _mm_acc(nc, ps[:],
                                [wg_t[:, c, ot * 128:(ot + 1) * 128] for c in range(4)],
                                [xn_blk[:, c, :] for c in range(4)])
                        sc = cb_t[:, ot:ot + 1]
                        if is_q:
                            nc.vector.scalar_tensor_tensor(
                                qrope[:, ot, :], ps[:], sc, cos_blk[:, ot, :],
                                ADD, MULT)
                            nc.vector.scalar_tensor_tensor(
                                qrope[:, 4 + ot, :], ps[:], sc, sin_blk[:, ot, :],
                                ADD, MULT)
                        elif ot < 8:
                            c2 = ot - 4
                            nc.vector.scalar_tensor_tensor(
                                krope[:, c2, :], ps[:], sc, cos_blk[:, c2, :],
                                ADD, MULT)
                            nc.vector.scalar_tensor_tensor(
                                krope[:, 4 + c2, :], ps[:], sc, sin_blk[:, c2, :],
                                ADD, MULT)
                        else:
                            c2 = ot - 8
                            nc.vector.tensor_scalar(
                                vn_blk[:, c2, :], ps[:], sc, None, ADD)

                    # in_proj
                    for o2 in range(4):
                        ps = mmp.tile([128, RB], DT.float32, tag="mm")
                        _mm_acc(nc, ps[:],
                                [wq_t[:, c, o2 * 128:(o2 + 1) * 128]
                                 for c in range(8)],
                                [qrope[:, c, :] for c in range(8)])
                        nc.vector.tensor_tensor(
                            q2_t[:, o2, r0:r0 + RB], ps[:], tq_blk[:, o2, :], ADD)
                    k2s = stg.tile([128, 4, RB], DT.bfloat16, tag="k2s",
                                   name="k2s")
                    for o2 in range(4):
                        ps = mmp.tile([128, RB], DT.float32, tag="mm")
                        _mm_acc(nc, ps[:],
                                [wk_t[:, c, o2 * 128:(o2 + 1) * 128] for c in range(8)],
                                [krope[:, c, :] for c in range(8)])
                        nc.vector.tensor_tensor(
                            k2s[:, o2, :], ps[:], tk_blk[:, o2, :], ADD)
                    nc.sync.dma_start(
                        out=kv_in[rb, 0, :].rearrange("(c p r) -> p c r",
                                                      p=128, r=RB),
                        in_=k2s[:])
                    # v in_proj: activations stationary -> row-major v2 [k, d]
                    v2s = stg.tile([128, 4, D], DT.bfloat16, tag="v2s", name="v2s")
                    for rc in range(RB // 128):
                        ps = mmp.tile([128, D], DT.float32, tag="mm")
                        _mm_acc(nc, ps[:],
                                [vn_blk[:, c, rc * 128:(rc + 1) * 128]
                                 for c in range(4)],
                                [wv_t[:, c, :] for c in range(4)])
                        nc.vector.tensor_tensor(
                            v2s[:, rc, :], ps[:], cv_t[:], ADD)
                    nc.sync.dma_start(
                        out=kv_in[rb, 1, :].rearrange("(j p d) -> p j d",
                                                      p=128, d=D),
                        in_=v2s[:])

                # Pair-wise K/V exchange, pipelined per block so the
                # gathers overlap the remaining blocks' compute. Key order
                # after each gather is [pair-even rows, pair-odd rows] on
                # BOTH cores, which is fine: softmax attention is
                # permutation-invariant over keys and each row carries its
                # own rope/bias.
                def emit_gather(rb):
                    nc.gpsimd.collective_compute(
                        "AllGather", mybir.AluOpType.bypass, replica_groups=RG,
                        ins=[kv_in[rb].opt()], outs=[kv_out[rb].opt()])
                    r0 = rb * RB
                    for half in range(2):
                        nc.sync.dma_start(
                            out=k2_t[:, :, half * SQ + r0:half * SQ + r0 + RB],
                            in_=kv_out[rb, 2 * half, :]
                            .rearrange("(c p r) -> p c r", p=128, r=RB))
                        nc.sync.dma_start(
                            out=v2_t[:, half * 16 + rb * 4:half * 16 + rb * 4 + 4, :],
                            in_=kv_out[rb, 2 * half + 1, :]
                            .rearrange("(j p d) -> p j d", p=128, d=D))

                emit_weight_loads()
                for it in range(NBL + 1):
                    if it < NBL:
                        emit_stats(it)
                    if it >= 1:
                        emit_main(it - 1)
                        emit_gather(it - 1)

            # ---------------- phase D: attention + out_proj ---------------
            # Per q-tile: 32 key-chunk iterations of {scores, exp, rowsum,
            # PV-accumulate}, then a tail {1/rowsum, normalize, out_proj}.
            # The tail of q-tile t is emitted after the first HEAD score/exp
            # groups of q-tile t+1 so TensorE never drains.
            HEAD = 22
            with tc.tile_pool(name="attn", bufs=2) as ap_, \
                 tc.tile_pool(name="exp", bufs=28) as ep, \
                 tc.tile_pool(name="ps_sc", bufs=3, space="PSUM") as scp, \
                 tc.tile_pool(name="ps_o", bufs=1, space="PSUM") as op_, \
                 tc.tile_pool(name="ps_rs", bufs=1, space="PSUM") as rsp:

                def emit_sc_exp(qt, j):
                    q0 = qt * 512
                    sc_ps = scp.tile([128, 512], DT.float32, tag="sc", name="sc_ps")
                    _mm_acc(nc, sc_ps[:],
                            [k2_t[:, c, j * 128:(j + 1) * 128] for c in range(4)],
                            [q2_t[:, c, q0:q0 + 512] for c in range(4)])
                    e = ep.tile([128, 512], DT.bfloat16, tag="e", name="e")
                    nc.scalar.activation(e[:], sc_ps[:],
                                         mybir.ActivationFunctionType.Exp,
                                         scale=1.0 / math.sqrt(D))
                    return e

                def emit_rs_pv(o_ps, rs_ps, e, j):
                    nc.tensor.matmul(rs_ps[:], ones_bf[:], e[:],
                                     start=(j == 0), stop=(j == NKC - 1))
                    for dt in range(4):
                        nc.tensor.matmul(
                            o_ps[dt][:], v2_t[:, j, dt * 128:(dt + 1) * 128], e[:],
                            start=(j == 0), stop=(j == NKC - 1))

                def emit_tail(qt, o_ps, rs_ps):
                    q0 = qt * 512
                    rinv_row = ap_.tile([1, 512], DT.float32, tag="rinv_row",
                                        name="rinv_row")
                    nc.vector.reciprocal(rinv_row[:], rs_ps[:])
                    rinv_bc = ap_.tile([128, 512], DT.float32, tag="rinv_bc",
                                       name="rinv_bc")
                    nc.sync.dma_start(out=rinv_bc[:], in_=_bcast_ap(rinv_row[:]))
                    on_t = ap_.tile([128, 4, 512], DT.bfloat16, tag="on", name="on_t")
                    for dt in range(4):
                        nc.vector.tensor_copy(on_t[:, dt, :], o_ps[dt][:])
                    for o3 in range(4):
                        fp = scp.tile([128, 512], DT.float32, tag="sc", name="fp")
                        _mm_acc(nc, fp[:],
                                [wo_t[:, c, o3 * 128:(o3 + 1) * 128] for c in range(4)],
                                [on_t[:, c, :] for c in range(4)])
                        fin = ap_.tile([128, 512], DT.float32, tag="fin", name="fin")
                        nc.vector.tensor_tensor(fin[:], fp[:], rinv_bc[:], MULT)
                        nc.vector.tensor_scalar(fin[:], fin[:], outb_t[:, o3:o3 + 1],
                                                None, ADD)
                        nc.sync.dma_start(
                            out=out[o3 * 128:(o3 + 1) * 128, q0:q0 + 512],
                            in_=fin[:])

                prev = None  # (qt, o_ps, rs_ps) awaiting tail emission
                for qt in range(NQT):
                    o_ps = [op_.tile([128, 512], DT.float32, tag=f"o{dt}",
                                     name=f"o_ps{dt}") for dt in range(4)]
                    rs_ps = rsp.tile([1, 512], DT.float32, tag="rs", name="rs_ps")
                    head_e = [emit_sc_exp(qt, j) for j in range(HEAD)]
                    if prev is not None:
                        emit_tail(*prev)
                    for j in range(HEAD):
                        emit_rs_pv(o_ps, rs_ps, head_e[j], j)
                    for j in range(HEAD, NKC):
                        e = emit_sc_exp(qt, j)
                        emit_rs_pv(o_ps, rs_ps, e, j)
                    prev = (qt, o_ps, rs_ps)
                emit_tail(*prev)
    nc.compile()
    return nc


_NC_CACHE = None


def _get_nc():
    global _NC_CACHE
    if _NC_CACHE is None:
        _NC_CACHE = build_nc()
    return _NC_CACHE


def _rope_tables():
    inv = 1.0 / (10000.0 ** (np.arange(0, D, 2, dtype=np.float64) / D))
    fr = np.arange(S, dtype=np.float64)[:, None] * inv[None, :]
    cos = np.repeat(np.cos(fr), 2, axis=-1)
    sin = np.repeat(np.sin(fr), 2, axis=-1)
    return cos, sin  # [S, D] float64


def _pack(a):
    """[D, R] feature-major -> [128, (R//RB)*4*RB] partition/block-major."""
    r = a.shape[1]
    nb = r // RB
    return np.ascontiguousarray(
        a.reshape(4, 128, nb, RB).transpose(1, 2, 0, 3).reshape(128, nb * 4 * RB))


def _packw(w):
    """[C*128, O] -> [128, C*O] partition-major weight packing."""
    c = w.shape[0] // 128
    o = w.shape[1]
    return np.ascontiguousarray(
        w.reshape(c, 128, o).transpose(1, 0, 2).reshape(128, c * o))


def _rot_vec(v):
    vp = v.reshape(-1, 2)
    return np.stack((-vp[:, 1], vp[:, 0]), axis=-1).reshape(-1)


def prep_in_maps(inputs):
    x = np.asarray(inputs["x"], np.float32)
    ln_g = np.asarray(inputs["ln_g"], np.float32)
    ln_b = np.asarray(inputs["ln_b"], np.float32)
    qkv_w = np.asarray(inputs["qkv_w"], np.float32)
    qkv_b = np.asarray(inputs["qkv_b"], np.float32)
    in_w = np.asarray(inputs["in_w"], np.float32)
    in_b = np.asarray(inputs["in_b"], np.float32)
    out_w = np.asarray(inputs["out_w"], np.float32)
    out_b = np.asarray(inputs["out_b"], np.float32)

    cos, sin = _rope_tables()

    # LN-fold: h = xhat * g + b ; qkv = h @ qkv_w.T + qkv_b
    #        = xhat @ (qkv_w * g).T + (b @ qkv_w.T + qkv_b)
    Wg = qkv_w * ln_g[None, :]
    cb_vec = ln_b @ qkv_w.T + qkv_b  # [1536]

    wq, wk, wv = np.split(in_w, 3, axis=0)
    bq, bk, bv = np.split(in_b, 3, axis=0)
    cbq, cbk, cbv = np.split(cb_vec, 3)

    # rope rotation matrix R: rot(q) = q @ R.T
    R = np.zeros((D, D), np.float32)
    for i in range(D // 2):
        R[2 * i, 2 * i + 1] = -1.0
        R[2 * i + 1, 2 * i] = 1.0

    wgT = _packw(Wg.T.astype(BF16))
    wqT = _packw(np.concatenate([wq.T, (wq @ R).T], 0).astype(BF16))
    wkT = _packw(np.concatenate([wk.T, (wk @ R).T], 0).astype(BF16))
    wvT = _packw(wv.T.astype(BF16))
    woT = _packw(out_w.T.astype(BF16))
    cb_t = np.ascontiguousarray(cb_vec.reshape(12, 128).T).astype(np.float32)
    outb_t = np.ascontiguousarray(out_b.reshape(4, 128).T).astype(np.float32)
    cv_vec = wv @ cbv + bv
    cv_t = np.broadcast_to(cv_vec[None, :], (128, D)).astype(np.float32).copy()

    rope_cbq = cbq[None, :] * cos + _rot_vec(cbq)[None, :] * sin        # [S, D] f64
    rope_cbk = cbk[None, :] * cos + _rot_vec(cbk)[None, :] * sin
    tq_full = (rope_cbq @ wq.T.astype(np.float64) + bq).astype(np.float32)  # [S, D]
    tk_full = (rope_cbk @ wk.T.astype(np.float64) + bk).astype(np.float32)

    in_maps = []
    for core in range(N_CORES):
        b, h = divmod(core, 2)
        pos = np.arange(h * SQ, (h + 1) * SQ)
        xs = x[b][pos]                                   # [SQ, D] own half
        in_maps.append({
            "xT": _pack(xs.T.astype(BF16)),
            "cosT": _pack(cos[pos].T.astype(BF16)),
            "sinT": _pack(sin[pos].T.astype(BF16)),
            "wgT": wgT, "wqT": wqT, "wkT": wkT, "wvT": wvT, "woT": woT,
            "tq": _pack(tq_full[pos].T.astype(BF16)),
            "tk": _pack(tk_full[pos].T.astype(BF16)),
            "cb": cb_t, "outb": outb_t, "cv": cv_t,
        })
    return in_maps


def assemble_out(results):
    out_full = np.zeros((B, S, D), np.float32)
    for core in range(N_CORES):
        b, h = divmod(core, 2)
        out_full[b, h * SQ:(h + 1) * SQ, :] = results[core]["out"].T
    return out_full


def kernel(**inputs):
    nc = _get_nc()
    in_maps = prep_in_maps(inputs)
    res = run_bass_kernel_spmd(nc, in_maps, core_ids=list(range(N_CORES)))
    return assemble_out(res.results)
